# revision 39
# baseline (speedup 1.0000x reference)
"""Multi-head attention (B=2, L=2048, D=1024, H=16) on 8 trn2 NeuronCores.

Sharding: tensor-parallel over heads - 2 heads per core. Each core computes
q/k/v projections for its 2 heads, the attention for those heads, and a
row-parallel partial of the output projection (transposed). The host sums
the 8 partials (the "all-reduce") and adds the biases that were folded out
of the device kernel (bv folded through Wo, plus bo).

Device schedule: the kernel alternates between the ACT engine's exp
stream (one [128,1024] exp per 128-column k-tile, ~1.29us each incl. sem
overhead, 128 k-tiles total = a hard ~165us floor) and a PE stream kept
just under it, so the exp stream runs essentially stall-free:

  - Attention runs as 8 single-head units of 16 k-tiles. Per k-tile the PE
    does two 512-col logits matmuls, one filler step, and a PV pair, all
    under ACT's exp. A second filler step per period costs ~390ns of
    stream stretch (measured), so filler stays at 1/period except unit 0's
    PV-free head.
  - PSUM (8 banks): logits double-buffer "pl" 2x[128,1024]f32 (4 banks) +
    PV accumulator "pv" (2 banks) + two [128,512] filler slots (2 banks).
  - PV lags SIX k-tiles: exp(k) completing releases both PV(k) and the
    pl-slot WAR for logits(k+2); the lag keeps released-but-queued PV work
    out of the release->logits->exp critical chain. The last six PVs of a
    unit plus the pv-draining epilogue carry over into the next unit.
  - P0 (pre-stream): kt rc0/rc1, vt rc0/rc1, qt rc0 full tiles + qt rc1
    half1 + qt rc2 half0 (chasing the rc2 DMA pieces). qt rc2 half1 rides
    unit 0's PV-free head in the idle pv banks (lazy pv-tile creation
    keeps the WAR chain clean); the other projections ride 1/period.
  - Input DMA alternates the sync and scalar HWDGE queues (piece 0 first,
    so kt0-c0 is gated only by one 256KB transfer after boot).
  - Softmax epilogue per unit is DVE-only (pv -> sbuf copy, then
    reciprocal_approx_fast + cross-partition swap DMA + one deferred
    normalize-mul), so ACT never switches activation tables. The LAST
    unit's epilogue instead uses ACT for the pv copy and gpsimd for the
    normalize-mul, keeping the tail's DVE free for out-proj drains.
  - va packing: [v|ones] for head 0, [ones|v] for head 1, so the PV matmul
    also produces the softmax denominator in the free half of the
    partitions (the ones columns ride in the stationary M dim for free).
  - Output partials are stored fp16 (halves HBM store traffic; host sums
    in float64), batched as [128,1024] tiles = one DMA per two psum
    drains. Tail out-projection: po tiles rotate 5 psum slots, each ost
    pair is drained by ONE engine with pairs alternating DVE/ACT (two
    pairs pipeline across engines, no cross-engine writes to one tile).
"""

import numpy as np
import ml_dtypes

import concourse.bass as bass
import concourse.mybir as mybir
import concourse.tile as tile
from concourse import bacc
from concourse.bass_utils import run_bass_kernel_spmd
from concourse.masks import make_identity

B, L, D, H = 2, 2048, 1024, 16
HD = D // H              # 64 head dim
N_CORES = 8
HPC = H // N_CORES       # 2 heads per core
DK = HPC * HD            # 128 local qkv feature dim
R = B * L                # 4096 rows
KC = D // 128            # 8 contraction chunks for the projections
NB = 1024                # q-block width (one attention unit)
NRC = R // NB            # 4 row chunks
NU = L // NB             # 2 attention units per batch per head
NKT = L // 128           # 16 k tiles per batch
NRT = R // 128           # 32 row tiles
SCALE = HD ** -0.5

BF16 = mybir.dt.bfloat16
F16 = mybir.dt.float16
F32 = mybir.dt.float32
Act = mybir.ActivationFunctionType

_BF16_NP = ml_dtypes.bfloat16


def _body(tc, nc, xt_d, wqt_d, wkt_d, wvt_d, bq_d, bk_d, wot_d, out_d):
    with (
        tc.tile_pool(name="consts", bufs=1) as constp,
        tc.tile_pool(name="bigs", bufs=1) as bigs,
        tc.tile_pool(name="work", bufs=1) as work,
        tc.tile_pool(name="psum", bufs=1, space="PSUM") as psum,
    ):
        def mm2(ps, lhsT, rhs, start, stop):
            # one weight load, two pipelined 512-wide matmuls (psum bank limit)
            for s in (slice(0, 512), slice(512, NB)):
                nc.tensor.matmul(ps[:, s], lhsT=lhsT, rhs=rhs[:, s], start=start, stop=stop)

        # ---- load weights / biases ----
        wq_sb = constp.tile([128, KC, DK], BF16)
        wk_sb = constp.tile([128, KC, DK], BF16)
        wv_sb = constp.tile([128, KC, DK], BF16)
        wot_sb = constp.tile([DK, D], BF16)
        bq_sb = constp.tile([DK, 1], F32)
        bk_sb = constp.tile([DK, 1], F32)
        ident = constp.tile([128, 128], BF16)
        zeros = constp.tile([128, 128], BF16)
        nc.gpsimd.memset(zeros[:], 0.0)
        # the projection weights go out concurrently on both HWDGE queues
        nc.sync.dma_start(out=wk_sb, in_=wkt_d[:])
        nc.scalar.dma_start(out=wv_sb, in_=wvt_d[:])
        nc.scalar.dma_start(out=wq_sb, in_=wqt_d[:])
        make_identity(nc, ident)

        # ---- load X.T ----
        # 32 column-major pieces: all 8 contraction chunks of row-chunk 0
        # first, then row-chunk 1, ... so each projection tile only waits for
        # its own columns (subtile deps) instead of the full 8MB transfer.
        # Pieces alternate between the sync and scalar DMA queues to halve
        # the ~600ns/issue serialization (the transfers themselves are close
        # to HBM-bandwidth-bound either way).
        xt_sb = [bigs.tile([128, R], BF16, name=f"xt{c}") for c in range(KC)]

        def xt_piece(c, rc, eng):
            cols = slice(rc * NB, (rc + 1) * NB)
            eng.dma_start(
                out=xt_sb[c][:, cols],
                in_=xt_d[c * 128 : (c + 1) * 128, cols],
            )

        for c in range(KC):
            xt_piece(c, 0, nc.sync if c % 2 == 0 else nc.scalar)
        nc.sync.dma_start(out=bk_sb, in_=bk_d[:])
        nc.scalar.dma_start(out=bq_sb, in_=bq_d[:])
        for c in range(KC):
            xt_piece(c, 1, nc.sync if c % 2 == 0 else nc.scalar)
        for c in range(KC):
            xt_piece(c, 2, nc.sync if c % 2 == 0 else nc.scalar)
        nc.scalar.dma_start(out=wot_sb, in_=wot_d[:])
        for c in range(KC):
            xt_piece(c, 3, nc.sync if c % 2 == 0 else nc.scalar)

        qt = bigs.tile([DK, R], BF16)
        kt = bigs.tile([DK, R], BF16)
        vt = bigs.tile([DK, R], BF16)
        yt = bigs.tile([DK, R], BF16)
        # va[h]: per 128-row k tile, [v_h | ones] for h0 and [ones | v_h] for
        # h1; the ones columns make the PV matmul also emit the softmax
        # denominator (h0: partitions 64:128, h1: partitions 0:64).
        va = [bigs.tile([128, R], BF16, name=f"va{h}") for h in range(HPC)]
        for h in range(HPC):
            nc.gpsimd.memset(va[h][:], 1.0)

        # ---- projection helpers ----
        def proj_drain(pp, dest, cols, bsb):
            if bsb is not None:
                nc.vector.tensor_scalar_add(out=dest[:, cols], in0=pp, scalar1=bsb)
            else:
                nc.vector.tensor_copy(out=dest[:, cols], in_=pp)

        def emit_proj_tile(wsb, bsb, dest, rc, dummies=False):
            # P0 only: full-width tile through the "pl" slots. `dummies`
            # interleaves warm-keepers so the PE p-state holds through the
            # xt-piece arrival waits.
            pp = psum.tile([128, NB], F32, tag="pl", bufs=2, name="pp")
            for c in range(KC):
                mm2(pp, wsb[:, c, :], xt_sb[c][:, rc * NB : (rc + 1) * NB],
                    start=(c == 0), stop=(c == KC - 1))
                if dummies:
                    nc.tensor.matmul(
                        warm[:, 0:512], lhsT=zeros, rhs=wk_sb[:, 0:4, :],
                        start=False, stop=False, skip_group_check=True,
                    )
            proj_drain(pp, dest, slice(rc * NB, (rc + 1) * NB), bsb)

        def emit_va_tile(t):
            # transpose one 128-row tile of vt into the va tiles
            pt = psum.tile([128, 128], BF16, tag="fil", bufs=2, name="pt")
            nc.tensor.transpose(pt, vt[:, t * 128 : (t + 1) * 128], ident)
            # h0 va block is [v | ones]; h1 va block is [ones | v]
            nc.vector.tensor_copy(
                out=va[0][:, t * 128 : t * 128 + HD], in_=pt[:, 0:HD]
            )
            nc.vector.tensor_copy(
                out=va[1][:, t * 128 + HD : (t + 1) * 128], in_=pt[:, HD:DK]
            )

        # ---- PE warmup ----
        # A dozen zero-weight matmuls streaming resident data bridge the gap
        # between the weights arriving and the first xt pieces landing, so
        # the p-state ramp reaches 2.4GHz before the projections start; kt0
        # then interleaves its own keep-warm dummies while chasing pieces.
        warm = psum.tile([128, NB], F32, tag="pl", bufs=2, name="warm")
        for di in range(12):
            nc.tensor.matmul(
                warm[:, 0:512], lhsT=zeros, rhs=wk_sb[:, 0:4, :],
                start=(di == 0), stop=False, skip_group_check=True,
            )

        # ---- P0: minimal prefix before the exp stream can start ----
        # (qt rc2 half1 rides unit 0's PV-free head in the idle pv banks;
        # the qt rc2 half0 steps chase the rc2 DMA pieces at the P0 tail)
        emit_proj_tile(wk_sb, bk_sb, kt, 0, dummies=True)
        emit_proj_tile(wv_sb, None, vt, 0)
        emit_proj_tile(wq_sb, bq_sb, qt, 0)
        emit_proj_tile(wk_sb, bk_sb, kt, 1)
        emit_proj_tile(wv_sb, None, vt, 1)

        # ---- filler step lists per attention unit ----
        # Half-major projection filler: one [128,512] half-tile (1 psum
        # bank) accumulates its 8 chunks then drains, so only ONE of the two
        # "fil" slots is pinned at a time (the other rotates pt / po tiles).
        def proj_steps(wsb, bsb, dest, rc, tags=("fil", "fil")):
            state = {}
            steps = []
            for half in (0, 1):
                cols = slice(rc * NB + half * 512, rc * NB + (half + 1) * 512)
                tg, tb = tags[half], (1 if tags[half] == "pv" else 2)
                for c in range(KC):
                    def step(c=c, cols=cols, last=(c == KC - 1), tg=tg, tb=tb):
                        if c == 0:
                            state["pp"] = psum.tile(
                                [128, 512], F32, tag=tg, bufs=tb, name="fp"
                            )
                        pp = state["pp"]
                        nc.tensor.matmul(
                            pp, lhsT=wsb[:, c, :], rhs=xt_sb[c][:, cols],
                            start=(c == 0), stop=(c == KC - 1),
                        )
                        if last:
                            proj_drain(pp, dest, cols, bsb)
                    steps.append(step)
            return steps

        ost_ref = {}
        po_ref = {}

        def emit_outproj_half(ofb, half, qc, copy_eng="dve", tag="fil",
                              store_eng=None, wide=False):
            # In-stream (wide=False): half-width matmul (1 psum bank) so the
            # fil-slot WAR waits on a short copy that fits under the exp
            # period; the halves share one [128,1024] fp16 staging tile and
            # go out as a single DMA. Tail (wide=True): both halves matmul
            # into one [128,1024] po and drain with a single copy + store,
            # halving the instruction/semaphore count.
            qh = slice(qc.start + half * 512, qc.start + (half + 1) * 512)
            if wide:
                if half == 0:
                    po_ref["t"] = psum.tile([128, NB], F32, tag=tag,
                                            bufs=(1 if tag != "pl" else 2),
                                            name="po")
                po = po_ref["t"][:, half * 512 : (half + 1) * 512]
            else:
                po = psum.tile([128, 512], F32, tag=tag,
                               bufs=(1 if tag == "pv" else 2), name="po")
            nc.tensor.matmul(
                po, lhsT=wot_sb[:, ofb * 128 : (ofb + 1) * 128],
                rhs=yt[:, qh], start=True, stop=True,
            )
            if half == 0:
                ost_ref["t"] = work.tile([128, NB], F16, tag="ost", bufs=6,
                                         name="ost")
            ost = ost_ref["t"]
            if wide:
                if half == 1:
                    src = po_ref["t"]
                    if copy_eng == "act":
                        nc.scalar.copy(out=ost, in_=src)
                    else:
                        nc.vector.tensor_copy(out=ost, in_=src)
            else:
                oh = slice(half * 512, (half + 1) * 512)
                if copy_eng == "act":
                    nc.scalar.copy(out=ost[:, oh], in_=po)
                else:
                    nc.vector.tensor_copy(out=ost[:, oh], in_=po)
            if half == 1:
                (store_eng or nc.sync).dma_start(
                    out=out_d[ofb * 128 : (ofb + 1) * 128, qc], in_=ost
                )

        def outproj_steps(pair, copy_engs=("dve", "dve"), tags=("fil", "fil"),
                          store_engs=(None,), wide=False):
            b, u = pair
            qc = slice(b * L + u * NB, b * L + (u + 1) * NB)
            items = []
            for i, (ofb, half) in enumerate(
                (ofb, half) for ofb in range(8) for half in (0, 1)
            ):
                items.append(
                    lambda ofb=ofb, half=half, i=i: emit_outproj_half(
                        ofb, half, qc, copy_engs[i % len(copy_engs)],
                        tags[i % len(tags)], store_engs[i % len(store_engs)],
                        wide,
                    )
                )
            return items

        # filler schedule by unit index (units: (b,u,h) h-inner).
        # Deadlines: vt rc1 halves by u0-k8/k12 (va t8..15); qt rc1 before
        # u2 (p32); kt rc2 + qt rc2 before u4 (p64); kt rc3 halves by u4-k8
        # and u4-k12; vt rc2 by u4-k2 (va-b1 jit); vt rc3 halves by u4-k12
        # and u5-k2; qt rc3 before u6 (p96). Out-proj of batch 0 rides u6/u7.
        sp_q1 = proj_steps(wq_sb, bq_sb, qt, 1)
        sp_q2 = proj_steps(wq_sb, bq_sb, qt, 2, tags=("fil", "pv"))
        sp_q3 = proj_steps(wq_sb, bq_sb, qt, 3)
        sp_k2 = proj_steps(wk_sb, bk_sb, kt, 2)
        sp_k3 = proj_steps(wk_sb, bk_sb, kt, 3)
        sp_v2 = proj_steps(wv_sb, None, vt, 2)
        sp_v3 = proj_steps(wv_sb, None, vt, 3)
        # P0 tail: qt rc1 half1, then qt rc2 half0 chasing the rc2 DMA
        # pieces, right before the stream starts
        for st in sp_q1[8:] + sp_q2[:8]:
            st()
        filler = {
            0: sp_q2[8:] + sp_q1[:8],    # qt rc2 half1 (pv banks) + qt rc1 h0
            1: sp_k2,                    # kt rc2
            2: sp_k3,                    # kt rc3
            3: sp_v2,                    # vt rc2
            4: sp_v3,                    # vt rc3
            5: sp_q3,                    # qt rc3
            6: outproj_steps((0, 0)),
            7: outproj_steps((0, 1)),
        }
        # one filler step per period is what the stream absorbs stretch-free
        # (measured: a second step costs ~390ns); the sole exception is unit
        # 0's PV-free head, where the qt rc2 half1 piece rides the idle pv
        # banks (so it doesn't fight the va transposes for the fil slots).
        allow = {
            0: [2, 2, 2, 2, 1, 1, 1, 1, 1, 1, 1, 1, 1, 1, 1, 1],
        }
        for ui_, st_ in filler.items():
            assert sum(allow.get(ui_, [1] * NKT)) >= len(st_), ui_
        # just-in-time va transposes: (unit, k-tile) -> va row tile.
        # vt rc2 drains during u3 (halves at k7/k15); vt rc3 during u4.
        va_jit = {}
        for t in range(NKT):      # batch-0 va built just-in-time in u0
            va_jit[(0, t)] = t    # (vt b0 fully drained during P0)
        for t in range(NKT, NRT):
            if t < 24:
                va_jit[(4, t - 16)] = t      # u4 k0..7 (vt rc2 from u3)
            elif t < 28:
                va_jit[(4, t - 15)] = t      # u4 k9..12, one period after the
            else:                            # vt rc3 half0 drain settles
                va_jit[(5, t - 27)] = t      # u5 k1..4 (vt rc3 half1)

        # ---- attention ----
        LAG = 6
        units = [(b, u, h) for b in (0, 1) for u in (0, 1) for h in range(HPC)]
        pending_mul = None   # deferred normalize-mul, staged by the epilogue
        carry = []           # closures from the previous unit, 1 per k-tile

        def emit_pending_mul(eng=None):
            nonlocal pending_mul
            if pending_mul is None:
                return
            h, yun, rr, qc = pending_mul
            rows = slice(0, HD) if h == 0 else slice(HD, 128)
            (eng or nc.vector).tensor_mul(
                out=yt[rows, qc], in0=yun[rows, :], in1=rr[rows, :]
            )
            pending_mul = None

        for ui, (b, u, h) in enumerate(units):
            qc = slice(b * L + u * NB, b * L + (u + 1) * NB)
            hr = slice(h * HD, (h + 1) * HD)
            steps = filler.get(ui, [])
            allow_k = allow.get(ui, [1] * NKT)
            si = 0

            # pv is created lazily at the first PV so pv-tagged filler
            # pieces emitted earlier in this unit take the banks first
            pvr = {}
            es = {}

            def emit_pv(j, b=b, h=h, pvr=pvr, es=es):
                if "t" not in pvr:
                    pvr["t"] = psum.tile([128, NB], F32, tag="pv", bufs=1,
                                         name="pv")
                tg = b * NKT + j
                mm2(pvr["t"], va[h][:, tg * 128 : (tg + 1) * 128], es.pop(j),
                    start=(j == 0), stop=(j == NKT - 1))

            for k in range(NKT):
                if (ui, k) in va_jit:
                    emit_va_tile(va_jit[(ui, k)])
                kcols = slice(b * L + k * 128, b * L + (k + 1) * 128)
                pl = psum.tile([128, NB], F32, tag="pl", bufs=2, name="pl")
                mm2(pl, kt[hr, kcols], qt[hr, qc], True, True)
                e = work.tile([128, NB], BF16, tag="exp", bufs=8, name="e")
                nc.scalar.activation(out=e, in_=pl, func=Act.Exp, scale=SCALE)
                es[k] = e
                # one carried item from the previous unit per k-tile
                if carry:
                    carry.pop(0)()
                # filler steps (per-period allowance)
                for _ in range(allow_k[k]):
                    if si < len(steps):
                        steps[si]()
                        si += 1
                if k >= LAG:
                    emit_pv(k - LAG)
                # The PE queue is saturated in steady state (cadence is
                # PE-bound): no warm-keepers needed. pv's first real write
                # (PV(0), start=True) resets it; until then nothing reads it.
                # the deferred mul of the previous unit (staged by the
                # carried epilogue at k7; by k11 its rr swap DMA is done)
                if k == 11:
                    emit_pending_mul()

            # ---- stage the carry: last LAG PVs + pv-draining epilogue ----
            # pv packing: h0 = [y (0:64); den (64:128)], h1 = [den; y].
            # The epilogue is split into half-width steps (4 carry items)
            # so its DVE time spreads over 4 periods instead of bursting
            # 2.3us at k6-k7 and backing up the out-proj drains.
            usb_ref = {}
            rs_ref = {}

            def make_usb_copy(half, pvr=pvr, usb_ref=usb_ref, ui=ui):
                def f():
                    if half == 0:
                        usb_ref["t"] = work.tile([128, NB], F32, tag="usb",
                                                 bufs=2, name="usb")
                    usb = usb_ref["t"]
                    cols = slice(half * 512, (half + 1) * 512)
                    if ui == len(units) - 1:
                        # runs post-stream: ACT is free, DVE is the tail's
                        # copy bottleneck
                        nc.scalar.copy(out=usb[:, cols], in_=pvr["t"][:, cols])
                    else:
                        nc.vector.tensor_copy(out=usb[:, cols],
                                              in_=pvr["t"][:, cols])
                return f

            def make_recip_swap(half, h=h, qc=qc, usb_ref=usb_ref,
                                rs_ref=rs_ref):
                def f():
                    nonlocal pending_mul
                    usb = usb_ref["t"]
                    yrows = slice(0, HD) if h == 0 else slice(HD, 128)
                    drows = slice(HD, 128) if h == 0 else slice(0, HD)
                    if half == 0:
                        rs_ref["rsw"] = work.tile([128, NB], F32, tag="rsw",
                                                  bufs=2, name="rsw")
                        rs_ref["rr"] = work.tile([128, NB], F32, tag="rr",
                                                 bufs=2, name="rr")
                    rsw, rr = rs_ref["rsw"], rs_ref["rr"]
                    cols = slice(half * 512, (half + 1) * 512)
                    # full-128-partition op: custom DVE ops silently drop
                    # writes when the AP has a non-zero partition base; the
                    # y-half lanes produce garbage that nothing reads
                    nc.vector.reciprocal_approx_fast(out=rsw[:, cols],
                                                     in_=usb[:, cols])
                    nc.sync.dma_start(out=rr[yrows, cols],
                                      in_=rsw[drows, cols])
                    if half == 1:
                        pending_mul = (h, usb, rr, qc)
                return f

            carry = [
                (lambda j=j, f=emit_pv: f(j)) for j in range(NKT - LAG, NKT)
            ] + [make_usb_copy(0), make_usb_copy(1),
                 make_recip_swap(0), make_recip_swap(1)]

        # ---- tail: flush the last unit, out-projection of batch 1 ----
        # Carry flushes first so the epilogue chain (usb on ACT -> recip on
        # DVE -> rr swap -> normalize-mul on gpsimd) heads its queues and
        # ungates the (1,1) group early; the out-projection pipeline then
        # runs copy-bound with DVE/ACT alternating. po tiles cycle through
        # fil x2 + both pl slots ((1,1) also rotates through the freed pv
        # slot); batched stores ride the sync queue.
        for item in carry:
            item()
        emit_pending_mul(eng=nc.gpsimd)
        # each ost pair is copied by ONE engine, with pairs alternating
        # between DVE and ACT: two pairs pipeline across the engines and no
        # tile sees cross-engine writes
        tail_stores = (nc.sync,)
        for step in outproj_steps((1, 0), ("dve", "dve", "act", "act"),
                                  ("fil", "fil", "pl", "pl", "pv"),
                                  tail_stores):
            step()
        for step in outproj_steps((1, 1), ("dve", "dve", "act", "act"),
                                  ("fil", "fil", "pl", "pl", "pv"),
                                  tail_stores):
            step()


def build_bass():
    nc = bacc.Bacc("TRN2", target_bir_lowering=False, debug=False)
    xt_d = nc.dram_tensor("xt", [D, R], BF16, kind="ExternalInput")
    wqt_d = nc.dram_tensor("wqt", [128, KC, DK], BF16, kind="ExternalInput")
    wkt_d = nc.dram_tensor("wkt", [128, KC, DK], BF16, kind="ExternalInput")
    wvt_d = nc.dram_tensor("wvt", [128, KC, DK], BF16, kind="ExternalInput")
    bq_d = nc.dram_tensor("bq", [DK, 1], F32, kind="ExternalInput")
    bk_d = nc.dram_tensor("bk", [DK, 1], F32, kind="ExternalInput")
    wot_d = nc.dram_tensor("wot", [DK, D], BF16, kind="ExternalInput")
    out_d = nc.dram_tensor("out", [D, R], F16, kind="ExternalOutput")
    with tile.TileContext(nc) as tc:
        _body(tc, nc, xt_d, wqt_d, wkt_d, wvt_d, bq_d, bk_d, wot_d, out_d)
    nc.compile()
    return nc


_NC = None


def _get_nc():
    global _NC
    if _NC is None:
        _NC = build_bass()
    return _NC


def prepare(inputs):
    """Full inputs -> (per-core in_maps, host-side bias constant)."""
    q = np.asarray(inputs["query"], np.float32)
    Wq = np.asarray(inputs["Wq"], np.float32)
    Wk = np.asarray(inputs["Wk"], np.float32)
    Wv = np.asarray(inputs["Wv"], np.float32)
    Wo = np.asarray(inputs["Wo"], np.float32)
    bq = np.asarray(inputs["bq"], np.float32)
    bk = np.asarray(inputs["bk"], np.float32)
    bv = np.asarray(inputs["bv"], np.float32)
    bo = np.asarray(inputs["bo"], np.float32)

    X = q.reshape(R, D)
    xt = np.ascontiguousarray(X.T).astype(_BF16_NP)

    def wslice(W, hs):
        # W[hs].T laid out [p, chunk, m]: in-feat within chunk, chunk, out-feat
        return np.ascontiguousarray(
            W[hs, :].T.reshape(KC, 128, DK).transpose(1, 0, 2)
        ).astype(_BF16_NP)

    in_maps = []
    const = bo.astype(np.float64).copy()
    for c in range(N_CORES):
        hs = slice(c * DK, (c + 1) * DK)
        const += Wo[:, hs].astype(np.float64) @ bv[hs].astype(np.float64)
        in_maps.append(
            {
                "xt": xt,
                "wqt": wslice(Wq, hs),
                "wkt": wslice(Wk, hs),
                "wvt": wslice(Wv, hs),
                "bq": np.ascontiguousarray(bq[hs].reshape(DK, 1)),
                "bk": np.ascontiguousarray(bk[hs].reshape(DK, 1)),
                "wot": np.ascontiguousarray(Wo[:, hs].T).astype(_BF16_NP),
            }
        )
    return in_maps, const


def finish(results, const):
    acc = np.zeros((D, R), np.float64)
    for r in results:
        acc += np.asarray(r["out"], np.float64)
    out = acc.T + const[None, :]
    return out.astype(np.float32).reshape(B, L, D)


def run(in_maps, trace=False, **kwargs):
    nc = _get_nc()
    return run_bass_kernel_spmd(nc, in_maps, list(range(N_CORES)), trace=trace, **kwargs)


def kernel(**inputs):
    in_maps, const = prepare(inputs)
    res = run(in_maps)
    return finish(res.results, const)
